# revision 101
# baseline (speedup 1.0000x reference)
"""Trainium2 Bass kernel for nn_MultiHeadAttention (B=2, S=2048, E=1024,
H=16, D=64) on 8 NeuronCores.

Sharding: core c -> (batch b = c//4, head-group g = c%4). Each core
computes Q/K/V projections for its batch restricted to its 4 heads
(column-parallel Wq/Wk/Wv), full attention for those heads, and a
row-parallel partial fc_out (the 256 local features of Wo). The host
sums the 4 partial outputs per batch (the row-parallel "all-reduce")
and adds the fc bias bo once per batch during that host reduction.

Device numerics: matmuls in bf16 except fc_out (float32r); all PSUM
accumulation fp32; softmax exp on the Scalar engine in fp32 from PSUM
with its output (attention weights in [0,1]) stored bf16.

Schedule: one software-pipelined stream built to keep the Scalar
engine's exp throughput (the hard floor: 4 x 2048 x 2048 exps per
core at 1 elem/cycle/lane) saturated from ~20us onward and the PE
dense enough to stay out of HAM throttle:

 - inputs are repacked host-side chunk-major so every slab DMA is a
   plain contiguous [128, 4096] transfer (128 descriptors, not 1024)
 - preamble: project K^T (full S) and Q^T chunk 0
 - per 512-wide q chunk qc, per head pair j: sweep the 16 key tiles;
   each slot issues the two heads' K=64 score matmuls back-to-back
   into disjoint PE row groups (tile_position (0,0)/(64,0), run
   concurrently), one [128,1024] pair Exp, and the DEPTH-delayed
   attention*V matmuls; V projection (first sweep), remaining Q^T
   chunks, and the previous chunk's fc_out groups are injected
   between slots as filler so the PE never idles while ACT works
 - each sweep's trailing o-matmuls and softmax-normalize chain are
   carried into the next sweep's early slots (cross-sweep software
   pipeline) so the PE queue never blocks on the ACT-paced tail.
"""

import numpy as np
from contextlib import ExitStack

import concourse.tile as tile
from concourse import bacc, mybir
from concourse.bass_utils import run_bass_kernel_spmd

F32R = mybir.dt.float32r
F32 = mybir.dt.float32
BF16 = mybir.dt.bfloat16
AF = mybir.ActivationFunctionType

B, S, E, H, D = 2, 2048, 1024, 16, 64
HL = 4            # heads per core
FL = HL * D       # local feature slice (256)
N_CORES = 8


def build_nc(S=2048, E=1024):
    T = E // 128       # emb k-tiles
    C = S // 512       # 512-wide seq chunks
    QC = S // 512      # 512-wide q chunks for phase B
    NKT = S // 128     # key tiles
    E2 = E // 512      # output column halves
    XW = C * T * 512   # packed x row length
    scale = 1.0 / (E ** 0.5)

    nc = bacc.Bacc("TRN2", target_bir_lowering=False, debug=False)

    xqT = nc.dram_tensor("xqT", [128, XW], BF16, kind="ExternalInput").ap()
    xkT = nc.dram_tensor("xkT", [128, XW], BF16, kind="ExternalInput").ap()
    xvT = nc.dram_tensor("xvT", [128, XW], BF16, kind="ExternalInput").ap()
    Wq = nc.dram_tensor("Wq", [128, (E // 128) * 256], BF16, kind="ExternalInput").ap()
    Wk = nc.dram_tensor("Wk", [128, (E // 128) * 256], BF16, kind="ExternalInput").ap()
    Wv = nc.dram_tensor("Wv", [128, (E // 128) * 260], BF16, kind="ExternalInput").ap()
    bq = nc.dram_tensor("bq", [1, 256], BF16, kind="ExternalInput").ap()
    bk = nc.dram_tensor("bk", [1, 256], BF16, kind="ExternalInput").ap()
    bv = nc.dram_tensor("bv", [1, 260], BF16, kind="ExternalInput").ap()
    WoT = nc.dram_tensor("WoT", [64, 4 * E], F32R, kind="ExternalInput").ap()
    ones = nc.dram_tensor("ones", [1, 512], BF16, kind="ExternalInput").ap()
    out = nc.dram_tensor("out", [S, E], F32, kind="ExternalOutput").ap()

    with tile.TileContext(nc) as tc, ExitStack() as ctx:
        const = ctx.enter_context(tc.tile_pool(name="const", bufs=1))
        persist = ctx.enter_context(tc.tile_pool(name="persist", bufs=1))
        xt_pool = ctx.enter_context(tc.tile_pool(name="xt", bufs=3))
        pt_pool = ctx.enter_context(tc.tile_pool(name="pt", bufs=12))
        raw_pool = ctx.enter_context(tc.tile_pool(name="raw", bufs=4))
        ot_pool = ctx.enter_context(tc.tile_pool(name="ot", bufs=9))
        rc_pool = ctx.enter_context(tc.tile_pool(name="rc", bufs=4))
        bc_pool = ctx.enter_context(tc.tile_pool(name="bc", bufs=4))
        os_pool = ctx.enter_context(tc.tile_pool(name="os", bufs=6))
        xv_pool = ctx.enter_context(tc.tile_pool(name="xv", bufs=2))
        psB_s = ctx.enter_context(tc.tile_pool(name="psB_s", bufs=2, space="PSUM"))
        psB_o = ctx.enter_context(tc.tile_pool(name="psB_o", bufs=2, space="PSUM"))
        psF = ctx.enter_context(tc.tile_pool(name="psF", bufs=2, space="PSUM"))

        # ---- constants to SBUF ----
        wq_sb = const.tile([128, T * 256], BF16)
        wk_sb = const.tile([128, T * 256], BF16)
        wv_sb = const.tile([128, T * 260], BF16)
        wo_sb = const.tile([64, 4 * E], F32R)
        bq_sb = const.tile([1, 256], BF16)
        bk_sb = const.tile([1, 256], BF16)
        bv_sb = const.tile([1, 260], BF16)
        on_sb = const.tile([1, 512], BF16)
        # All constants go on the ACT DMA queue (wk first: K-projection
        # starts the kernel); the SP queue carries only input slabs so
        # the first K slab lands as early as possible.
        nc.scalar.dma_start(wk_sb[:], Wk)
        nc.scalar.dma_start(bk_sb[:], bk)
        nc.scalar.dma_start(on_sb[:], ones)
        nc.scalar.dma_start(wq_sb[:], Wq)
        nc.scalar.dma_start(bq_sb[:], bq)
        nc.scalar.dma_start(bv_sb[:], bv)
        # wv/wo (and later the V slabs) are gated behind the warm-up so
        # their transfers don't steal DMA bandwidth from the K0/Q0
        # slabs that gate the first attention sweep
        late_dmas = [nc.scalar.dma_start(wv_sb[:], Wv),
                     nc.scalar.dma_start(wo_sb[:], WoT)]

        # PE warm-up: dense back-to-back bf16 matmuls while the first DMAs
        # land, so the HAM un-throttles the PE clock before real work. The
        # warm-up exp also pre-loads the ACT exp table-set.
        with tc.tile_pool(name="wu", bufs=1) as wu_pool:
            wu = wu_pool.tile([128, 640], BF16)
            nc.gpsimd.memset(wu[:], 0.0)
            wux = wu_pool.tile([1, 32], F32, name="wux")
            nc.scalar.activation(wux[:], wu[0:1, 0:32], AF.Exp, scale=1.0)
            for i in range(28):
                wp = psF.tile([128, 512], F32, tag="psf", name="wup")
                wu_last = nc.tensor.matmul(wp[:], wu[:, 0:128],
                                           wu[:, 128:640],
                                           start=True, stop=True)
            for dma in late_dmas:
                tile.add_dep_helper(dma.ins, wu_last.ins,
                                    reason="defer bulk weight DMA")

        qt_sb = [persist.tile([128, S], BF16, tag=f"qt{j}", name=f"qt{j}")
                 for j in range(2)]
        kt_sb = [persist.tile([128, S], BF16, tag=f"kt{j}", name=f"kt{j}")
                 for j in range(2)]
        v_sb = persist.tile([128, NKT * 260], BF16, tag="v")

        # ---- projection emitters ----
        def emit_slab_dma(x_dram, c, pool=None, eng=None):
            slab = (pool or xt_pool).tile([128, T * 512], BF16, tag="slab")
            (eng or nc.sync).dma_start(
                slab[:], x_dram[:, c * T * 512 : (c + 1) * T * 512])
            return slab

        def emit_qk_steps(slab, w_sb, b_sb, dst, c, j):
            """One Q/K psum group as 3 filler steps (3+3+3 matmuls)."""
            ps = psF.tile([128, 512], F32, tag="psf", name="psqk")

            def step(t0, t1, ps=ps):
                for t in range(t0, t1):
                    nc.tensor.matmul(
                        ps[:],
                        w_sb[:, t * 256 + j * 128 : t * 256 + j * 128 + 128],
                        slab[:, t * 512 : (t + 1) * 512],
                        start=(t == 0), stop=False,
                    )
                if t1 == T:
                    nc.tensor.matmul(
                        ps[:], b_sb[:, j * 128 : (j + 1) * 128],
                        on_sb[:, 0:512], start=False, stop=True,
                    )
                    nc.vector.tensor_copy(
                        dst[:, c * 512 : (c + 1) * 512], ps[:]
                    )

            return [lambda a=a, b=b: step(a, b) for a, b in
                    [(0, 2), (2, 4), (4, 6), (6, T)]]

        def emit_v_steps(slab, c, s):
            """One V psum group split into 3 filler steps."""
            ps = psF.tile([128, 512], F32, tag="psf", name="psv")

            def step(t0, t1, ps=ps):
                for t in range(t0, t1):
                    nc.tensor.matmul(
                        ps[:, 0:260],
                        slab[:, t * 512 + s * 128 : t * 512 + s * 128 + 128],
                        wv_sb[:, t * 260 : (t + 1) * 260],
                        start=(t == 0), stop=False,
                    )
                if t1 == T:
                    nc.tensor.matmul(
                        ps[:, 0:260], on_sb[:, 0:128], bv_sb[:],
                        start=False, stop=True,
                    )
                    nc.vector.tensor_copy(
                        v_sb[:, (4 * c + s) * 260 : (4 * c + s + 1) * 260],
                        ps[:, 0:260],
                    )

            return [lambda a=a, b=b: step(a, b) for a, b in
                    [(0, 3), (3, 6), (6, T)]]

        # ---- preamble: K^T chunk 0 + Q^T chunk 0 head pair 0; the
        # rest (K^T 1-3, Q0 pair 1, all of V) is injected as fillers
        # into the first sweep, ordered against its consumption ----
        k0slab = emit_slab_dma(xkT, 0)
        q0slab = emit_slab_dma(xqT, 0)
        k1slab = emit_slab_dma(xkT, 1)
        for j in range(2):
            for f in emit_qk_steps(k0slab, wk_sb, bk_sb, kt_sb[j], 0, j):
                f()
        for f in emit_qk_steps(q0slab, wq_sb, bq_sb, qt_sb[0], 0, 0):
            f()

        # ---- fc_out group emitter (4 K=64 matmuls + drain + DMA) ----
        dma_q = [0]

        def emit_fc_group(qc, ot_tiles, ss, e2, o_sb, anchor=None,
                          act_drain=False):
            ps_f = psF.tile([128, 512], F32, tag="psf", name="psf")
            for h in range(4):
                mm = nc.tensor.matmul(
                    ps_f[:],
                    ot_tiles[h][:, ss * 128 : (ss + 1) * 128],
                    wo_sb[:, h * E + e2 * 512 : h * E + e2 * 512 + 512],
                    start=(h == 0), stop=(h == 3),
                )
                if h == 0 and anchor is not None:
                    tile.add_dep_helper(
                        mm.ins, anchor.ins,
                        reason="fc_out deferred behind later attention",
                    )
            nc.vector.tensor_copy(
                o_sb[:, e2 * 512 : (e2 + 1) * 512], ps_f[:]
            )
            if e2 == E2 - 1:
                r0 = qc * 512 + ss * 128
                nc.sync.dma_start(out[r0 : r0 + 128, :], o_sb[:])

        def emit_fc_half(qc, ot_tiles, ss, e2, o_sb, h0, anchor=None):
            """Half an fc_out group (2 heads). The h0==0 half can run
            during the final sweep (only the first head pair's ot is
            needed); the h0==2 half adds on top at the tail."""
            ps_f = psF.tile([128, 512], F32, tag="psf", name="psfh")
            for h in (h0, h0 + 1):
                mm = nc.tensor.matmul(
                    ps_f[:],
                    ot_tiles[h][:, ss * 128 : (ss + 1) * 128],
                    wo_sb[:, h * E + e2 * 512 : h * E + e2 * 512 + 512],
                    start=(h == h0), stop=(h == h0 + 1),
                )
                if h == h0 and anchor is not None:
                    tile.add_dep_helper(
                        mm.ins, anchor.ins,
                        reason="fc_out deferred behind later attention",
                    )
            osl = o_sb[:, e2 * 512 : (e2 + 1) * 512]
            if h0 == 0:
                nc.vector.tensor_copy(osl, ps_f[:])
            else:
                nc.vector.tensor_add(osl, osl, ps_f[:])
                if e2 == E2 - 1:
                    r0 = qc * 512 + ss * 128
                    # tail: both DMA queues in parallel, half rows each
                    nc.sync.dma_start(out[r0 : r0 + 64, :], o_sb[0:64, :])
                    nc.scalar.dma_start(out[r0 + 64 : r0 + 128, :],
                                        o_sb[64:128, :])

        # ---- phase B: attention sweeps, cross-sweep pipelined ----
        DEPTH = 5  # o-matmuls trail s/exp by this many k-tiles
        last_s_mm = [None]
        fillers = []      # pending filler closures (V/Q projections)
        fc_fillers = []   # pending fc groups (gated later: their ot
                          # inputs come from the previous sweep's
                          # normalize chain, ~5us of DVE/gpsimd latency)
        pending = []      # previous sweep's trailing o's + raw drains
        rest_queue = []   # deferred reciprocal/normalize chains

        def inject(n):
            k = min(n, len(fillers))
            for _ in range(k):
                fillers.pop(0)()
            return k

        def inject_fc(n):
            k = min(n, len(fc_fillers))
            for _ in range(k):
                fc_fillers.pop(0)()
            return k

        def keep_warm(n=2):
            """Dummy matmuls on resident tiles: deny the HAM an idle
            window when no real filler work is available this slot."""
            for _ in range(n):
                wp = psF.tile([128, 512], F32, tag="psf", name="warm")
                nc.tensor.matmul(wp[:], kt_sb[0][:, 0:128],
                                 qt_sb[0][:, 0:512], start=True, stop=True)

        # First-sweep filler inventory, ordered against consumption
        # (s needs K chunk c by slot 4c; o, DEPTH=8-delayed, needs V
        # chunk c by slot 4c+8; pair-1 work by the second sweep) and
        # against the slab rings' DMA readiness.
        k2slab = emit_slab_dma(xkT, 2)
        k3slab = emit_slab_dma(xkT, 3)
        # V slabs ride the ACT DMA queue, gated behind the warm-up so
        # they don't contend with the startup-critical K/Q slabs
        vslabs = []
        for c in range(C):
            slab = xv_pool.tile([128, T * 512], BF16, tag="slab")
            vd = nc.scalar.dma_start(
                slab[:], xvT[:, c * T * 512 : (c + 1) * T * 512])
            tile.add_dep_helper(vd.ins, wu_last.ins,
                                reason="defer V slab DMA")
            vslabs.append(slab)

        def v_units(c):
            us = []
            for sblk in range(4):
                us.extend(emit_v_steps(vslabs[c], c, sblk))
            return us

        fillers.extend(emit_qk_steps(k1slab, wk_sb, bk_sb, kt_sb[0], 1, 0))
        fillers.extend(emit_qk_steps(k2slab, wk_sb, bk_sb, kt_sb[0], 2, 0))
        fillers.extend(v_units(0))
        fillers.extend(emit_qk_steps(q0slab, wq_sb, bq_sb, qt_sb[1], 0, 1))
        fillers.extend(v_units(1))
        fillers.extend(emit_qk_steps(k1slab, wk_sb, bk_sb, kt_sb[1], 1, 1))
        fillers.extend(emit_qk_steps(k2slab, wk_sb, bk_sb, kt_sb[1], 2, 1))
        fillers.extend(v_units(2))
        fillers.extend(emit_qk_steps(k3slab, wk_sb, bk_sb, kt_sb[0], 3, 0))
        fillers.extend(emit_qk_steps(k3slab, wk_sb, bk_sb, kt_sb[1], 3, 1))
        fillers.extend(v_units(3))

        ot_store = {}
        sweeps = [(qc, j) for qc in range(QC) for j in range(2)]
        for w, (qc, j) in enumerate(sweeps):
            # filler inventory for this sweep
            if j == 0 and qc + 1 < QC:
                qslab = emit_slab_dma(xqT, qc + 1)
                for jj in range(2):
                    fillers.extend(
                        emit_qk_steps(qslab, wq_sb, bq_sb,
                                      qt_sb[jj], qc + 1, jj))
            if qc > 0:
                for ss in (0, 1) if j == 0 else (2, 3):
                    o_sb = os_pool.tile([128, E], F32, tag="osb",
                                        name="osb")
                    for e2 in range(E2):
                        fc_fillers.append(
                            lambda q=qc - 1, s=ss, e=e2, ob=o_sb:
                            emit_fc_group(q, ot_store[q], s, e, ob,
                                          anchor=last_s_mm[0]))
            if w == len(sweeps) - 1:
                # last chunk: the first head pair's fc halves can run
                # inside this sweep (their ot comes from this chunk's
                # j=0 normalize); the other halves add on at the tail
                tail_osb = []
                for ss in range(4):
                    o_sb = os_pool.tile([128, E], F32, tag="osb",
                                        name="osb")
                    tail_osb.append(o_sb)
                    for e2 in range(E2):
                        fc_fillers.append(
                            lambda s=ss, e=e2, ob=o_sb:
                            emit_fc_half(QC - 1, ot_store[QC - 1], s, e,
                                         ob, 0, anchor=last_s_mm[0]))

            po = [psB_o.tile([65, 512], F32, tag="po", name=f"po{e}")
                  for e in range(2)]
            pts = {}

            def emit_o(kt, po=po, pts=pts, j=j):
                for e in range(2):
                    h = 2 * j + e
                    nc.tensor.matmul(
                        po[e][:],
                        v_sb[:, kt * 260 + 65 * h : kt * 260 + 65 * h + 65],
                        pts[kt][:, e * 512 : (e + 1) * 512],
                        start=(kt == 0), stop=(kt == NKT - 1),
                    )
                del pts[kt]

            def make_normalize(po=po, qc=qc, j=j):
                """Split normalize: the raw drains pop right behind the
                trailing o's (freeing the po PSUM ring fast), while the
                slow reciprocal chain is deferred so it never convoys
                the DVE queue at a sweep boundary."""
                raws = []

                def norm_raw():
                    for e in range(2):
                        raw = raw_pool.tile([65, 512], F32, tag="raw",
                                            name="raw")
                        nc.vector.tensor_copy(raw[:], po[e][:])
                        raws.append(raw)

                def norm_rest():
                    ots = ot_store.setdefault(qc, {})
                    for e in range(2):
                        h = 2 * j + e
                        raw = raws[e]
                        rc = rc_pool.tile([1, 512], F32, tag="rc", name="rc")
                        with nc.allow_low_precision(reason="denom recip"):
                            nc.vector.reciprocal(rc[:], raw[64:65, :])
                        bc = bc_pool.tile([64, 512], F32, tag="bc", name="bc")
                        nc.gpsimd.partition_broadcast(bc[:], rc[:])
                        ot = ot_pool.tile([64, 512], F32R, tag="ot",
                                          name="ot")
                        nc.vector.tensor_mul(ot[:], raw[0:64, :], bc[:])
                        ots[h] = ot
                return norm_raw, norm_rest

            if w == 0:
                d = 8      # V chunks arrive as fillers; o trails more
            elif w == len(sweeps) - 1:
                d = 2      # shorten the final ACT-paced tail
            else:
                d = DEPTH
            for kt in range(NKT):
                ps_s = psB_s.tile([128, 1024], F32, tag="pss", name="pss")
                # the two heads of pair j land in disjoint PE row
                # groups (base partitions 0 / 64) and disjoint PSUM
                # banks, so these run concurrently in the array
                for e in range(2):
                    last_s_mm[0] = nc.tensor.matmul(
                        ps_s[:, e * 512 : (e + 1) * 512],
                        kt_sb[j][64 * e : 64 * e + 64,
                                 kt * 128 : (kt + 1) * 128],
                        qt_sb[j][64 * e : 64 * e + 64,
                                 qc * 512 : (qc + 1) * 512],
                        start=True, stop=True,
                    )
                pt = pt_pool.tile([128, 1024], BF16, tag="pt", name="pt")
                nc.scalar.activation(pt[:], ps_s[:], AF.Exp, scale=scale)
                pts[kt] = pt
                if kt < d:
                    for _ in range(2 if pending else 0):
                        if pending:
                            pending.pop(0)()
                else:
                    while pending:   # raw drains must precede o(0)'s WAR
                        pending.pop(0)()
                    emit_o(kt - d)
                if kt == 6 and rest_queue:
                    rest_queue.pop(0)()
                inject(5 if w == 0 and kt < 12 else (4 if w == 0 else 2))
                if j == 0 or w == len(sweeps) - 1:
                    if kt >= 10:
                        inject_fc(2)
                elif kt >= 2:
                    inject_fc(1)

            pending = [lambda kt=kt, f=emit_o: f(kt)
                       for kt in range(NKT - d, NKT)]
            if w < len(sweeps) - 1:
                nr, nrest = make_normalize()
                pending.append(nr)
                rest_queue.append(nrest)
            else:
                last_norm = (po, qc, j)

        # tail: flush trailing work, then the last pair's normalize in
        # 128-column blocks interleaved with its fc halves, so the fc
        # matmuls start ~1.5us after the last exp instead of waiting
        # for two full-width 3.3us reciprocals
        while pending:
            pending.pop(0)()
        while rest_queue:
            rest_queue.pop(0)()
        while fillers:
            inject(len(fillers))
        while fc_fillers:
            inject_fc(len(fc_fillers))
        po_l, qc_l, j_l = last_norm
        ots_l = ot_store.setdefault(qc_l, {})
        raws_l = []
        for e in range(2):
            raw = raw_pool.tile([65, 512], F32, tag="raw", name="rawt")
            nc.vector.tensor_copy(raw[:], po_l[e][:])
            raws_l.append(raw)
            ots_l[2 * j_l + e] = ot_pool.tile([64, 512], F32R, tag="ot",
                                              name="ott")
        for ss in range(4):
            cs = slice(ss * 128, ss * 128 + 128)
            for e in range(2):
                rcb = rc_pool.tile([1, 128], F32, tag="rcb", name="rcb")
                with nc.allow_low_precision(reason="denom recip"):
                    nc.vector.reciprocal(rcb[:], raws_l[e][64:65, cs])
                bcb = bc_pool.tile([64, 128], F32, tag="bcb", name="bcb")
                nc.gpsimd.partition_broadcast(bcb[:], rcb[:])
                nc.vector.tensor_mul(ots_l[2 * j_l + e][:, cs],
                                     raws_l[e][0:64, cs], bcb[:])
            for e2 in range(E2):
                emit_fc_half(QC - 1, ots_l, ss, e2, tail_osb[ss], 2)

    nc.compile()
    return nc

_NC_CACHE = [None]


def _get_nc():
    if _NC_CACHE[0] is None:
        _NC_CACHE[0] = build_nc(S=S, E=E)
    return _NC_CACHE[0]


def _pack_w_ktile(W):
    """[E, F] -> [128, (E//128)*F] with k-tile-major packing: column
    block t holds W[t*128:(t+1)*128, :]."""
    E_, F_ = W.shape
    T_ = E_ // 128
    return np.ascontiguousarray(
        W.reshape(T_, 128, F_).transpose(1, 0, 2).reshape(128, T_ * F_))


def _pack_x(xT, S=2048, E=1024):
    """[E, S] f32 -> [128, C*T*512] chunk-major so each 512-seq slab is
    one contiguous [128, T*512] DMA."""
    T_ = E // 128
    C_ = S // 512
    return np.ascontiguousarray(
        xT.reshape(T_, 128, C_, 512).transpose(1, 2, 0, 3).reshape(
            128, C_ * T_ * 512))


def make_in_maps(query, key, value, Wq, bq, Wk, bk, Wv, bv, Wo, bo):
    """Shard the full inputs into the 8 per-core input dicts."""
    bf = mybir.dt.np(BF16)
    f32 = np.float32
    query = np.asarray(query, f32)
    key = np.asarray(key, f32)
    value = np.asarray(value, f32)
    Wq, bq = np.asarray(Wq, f32), np.asarray(bq, f32)
    Wk, bk = np.asarray(Wk, f32), np.asarray(bk, f32)
    Wv, bv = np.asarray(Wv, f32), np.asarray(bv, f32)
    Wo, bo = np.asarray(Wo, f32), np.asarray(bo, f32)

    xT = {}
    for b in range(B):
        xT[b] = (
            _pack_x(query[b].T).astype(bf),
            _pack_x(key[b].T).astype(bf),
            _pack_x(value[b].T).astype(bf),
        )

    ones = np.ones((1, 512), bf)
    in_maps = []
    for c in range(N_CORES):
        b, g = c // 4, c % 4
        fs = slice(FL * g, FL * g + FL)
        # projection weights: reference computes x @ W.T, so the device
        # weight matrix is W.T's column slice = W[fs, :].T  [E, FL]
        wq_c = np.ascontiguousarray(Wq[fs, :].T)
        wk_c = np.ascontiguousarray(Wk[fs, :].T)
        wv_c = np.ascontiguousarray(Wv[fs, :].T)
        # V with interleaved ones-columns (via the bias row)
        wv_pack = np.zeros((E, HL * 65), f32)
        bv_pack = np.zeros((1, HL * 65), f32)
        bq_c = bq[fs][None, :]
        bk_c = bk[fs][None, :]
        for h in range(HL):
            wv_pack[:, 65 * h : 65 * h + 64] = wv_c[:, 64 * h : 64 * h + 64]
            bv_pack[0, 65 * h : 65 * h + 64] = bv[fs][64 * h : 64 * h + 64]
            bv_pack[0, 65 * h + 64] = 1.0
        # fc_out rows for the local features (bias bo is added on the
        # host during the cross-core reduction)
        wot = np.zeros((64, HL * E), f32)
        for h in range(HL):
            wot[:, E * h : E * h + E] = Wo[:, FL * g + 64 * h : FL * g + 64 * h + 64].T
        in_maps.append({
            "xqT": xT[b][0], "xkT": xT[b][1], "xvT": xT[b][2],
            "Wq": _pack_w_ktile(wq_c).astype(bf),
            "Wk": _pack_w_ktile(wk_c).astype(bf),
            "Wv": _pack_w_ktile(wv_pack).astype(bf),
            "bq": bq_c.astype(bf), "bk": bk_c.astype(bf),
            "bv": bv_pack.astype(bf),
            "WoT": wot,
            "ones": ones,
        })
    return in_maps


def assemble_output(results, bo):
    """Sum the row-parallel partial fc_out results per batch, add bo."""
    out = np.empty((B, S, E), np.float32)
    for b in range(B):
        acc = results[4 * b]["out"].astype(np.float32).copy()
        for g in range(1, 4):
            acc += results[4 * b + g]["out"]
        out[b] = acc + bo[None, :]
    return out


def kernel(query, key, value, Wq, bq, Wk, bk, Wv, bv, Wo, bo, **run_kwargs):
    nc = _get_nc()
    in_maps = make_in_maps(query, key, value, Wq, bq, Wk, bk, Wv, bv, Wo, bo)
    res = run_bass_kernel_spmd(nc, in_maps, core_ids=list(range(N_CORES)),
                               **run_kwargs)
    out = assemble_output(res.results, np.asarray(bo, np.float32))
    kernel.last_result = res
    return out


# revision 102
# speedup vs baseline: 1.0007x; 1.0007x over previous
"""Trainium2 Bass kernel for nn_MultiHeadAttention (B=2, S=2048, E=1024,
H=16, D=64) on 8 NeuronCores.

Sharding: core c -> (batch b = c//4, head-group g = c%4). Each core
computes Q/K/V projections for its batch restricted to its 4 heads
(column-parallel Wq/Wk/Wv), full attention for those heads, and a
row-parallel partial fc_out (the 256 local features of Wo). The host
sums the 4 partial outputs per batch (the row-parallel "all-reduce")
and adds the fc bias bo once per batch during that host reduction.

Device numerics: matmuls in bf16 except fc_out (float32r); all PSUM
accumulation fp32; softmax exp on the Scalar engine in fp32 from PSUM
with its output (attention weights in [0,1]) stored bf16.

Schedule: one software-pipelined stream built to keep the Scalar
engine's exp throughput (the hard floor: 4 x 2048 x 2048 exps per
core at 1 elem/cycle/lane) saturated from ~20us onward and the PE
dense enough to stay out of HAM throttle:

 - inputs are repacked host-side chunk-major so every slab DMA is a
   plain contiguous [128, 4096] transfer (128 descriptors, not 1024)
 - preamble: project K^T (full S) and Q^T chunk 0
 - per 512-wide q chunk qc, per head pair j: sweep the 16 key tiles;
   each slot issues the two heads' K=64 score matmuls back-to-back
   into disjoint PE row groups (tile_position (0,0)/(64,0), run
   concurrently), one [128,1024] pair Exp, and the DEPTH-delayed
   attention*V matmuls; V projection (first sweep), remaining Q^T
   chunks, and the previous chunk's fc_out groups are injected
   between slots as filler so the PE never idles while ACT works
 - each sweep's trailing o-matmuls and softmax-normalize chain are
   carried into the next sweep's early slots (cross-sweep software
   pipeline) so the PE queue never blocks on the ACT-paced tail.
"""

import numpy as np
from contextlib import ExitStack

import concourse.tile as tile
from concourse import bacc, mybir
from concourse.bass_utils import run_bass_kernel_spmd

F32R = mybir.dt.float32r
F32 = mybir.dt.float32
BF16 = mybir.dt.bfloat16
AF = mybir.ActivationFunctionType

B, S, E, H, D = 2, 2048, 1024, 16, 64
HL = 4            # heads per core
FL = HL * D       # local feature slice (256)
N_CORES = 8


def build_nc(S=2048, E=1024):
    T = E // 128       # emb k-tiles
    C = S // 512       # 512-wide seq chunks
    QC = S // 512      # 512-wide q chunks for phase B
    NKT = S // 128     # key tiles
    E2 = E // 512      # output column halves
    XW = C * T * 512   # packed x row length
    scale = 1.0 / (E ** 0.5)

    nc = bacc.Bacc("TRN2", target_bir_lowering=False, debug=False)

    xqT = nc.dram_tensor("xqT", [128, XW], BF16, kind="ExternalInput").ap()
    xkT = nc.dram_tensor("xkT", [128, XW], BF16, kind="ExternalInput").ap()
    xvT = nc.dram_tensor("xvT", [128, XW], BF16, kind="ExternalInput").ap()
    Wq = nc.dram_tensor("Wq", [128, (E // 128) * 256], BF16, kind="ExternalInput").ap()
    Wk = nc.dram_tensor("Wk", [128, (E // 128) * 256], BF16, kind="ExternalInput").ap()
    Wv = nc.dram_tensor("Wv", [128, (E // 128) * 260], BF16, kind="ExternalInput").ap()
    bq = nc.dram_tensor("bq", [1, 256], BF16, kind="ExternalInput").ap()
    bk = nc.dram_tensor("bk", [1, 256], BF16, kind="ExternalInput").ap()
    bv = nc.dram_tensor("bv", [1, 260], BF16, kind="ExternalInput").ap()
    WoT = nc.dram_tensor("WoT", [64, 4 * E], F32R, kind="ExternalInput").ap()
    ones = nc.dram_tensor("ones", [1, 512], BF16, kind="ExternalInput").ap()
    out = nc.dram_tensor("out", [S, E], F32, kind="ExternalOutput").ap()

    with tile.TileContext(nc) as tc, ExitStack() as ctx:
        const = ctx.enter_context(tc.tile_pool(name="const", bufs=1))
        persist = ctx.enter_context(tc.tile_pool(name="persist", bufs=1))
        xt_pool = ctx.enter_context(tc.tile_pool(name="xt", bufs=3))
        pt_pool = ctx.enter_context(tc.tile_pool(name="pt", bufs=12))
        raw_pool = ctx.enter_context(tc.tile_pool(name="raw", bufs=4))
        ot_pool = ctx.enter_context(tc.tile_pool(name="ot", bufs=9))
        rc_pool = ctx.enter_context(tc.tile_pool(name="rc", bufs=4))
        bc_pool = ctx.enter_context(tc.tile_pool(name="bc", bufs=4))
        os_pool = ctx.enter_context(tc.tile_pool(name="os", bufs=6))
        xv_pool = ctx.enter_context(tc.tile_pool(name="xv", bufs=2))
        psB_s = ctx.enter_context(tc.tile_pool(name="psB_s", bufs=2, space="PSUM"))
        psB_o = ctx.enter_context(tc.tile_pool(name="psB_o", bufs=2, space="PSUM"))
        psF = ctx.enter_context(tc.tile_pool(name="psF", bufs=2, space="PSUM"))

        # ---- constants to SBUF ----
        wq_sb = const.tile([128, T * 256], BF16)
        wk_sb = const.tile([128, T * 256], BF16)
        wv_sb = const.tile([128, T * 260], BF16)
        wo_sb = const.tile([64, 4 * E], F32R)
        bq_sb = const.tile([1, 256], BF16)
        bk_sb = const.tile([1, 256], BF16)
        bv_sb = const.tile([1, 260], BF16)
        on_sb = const.tile([1, 512], BF16)
        # All constants go on the ACT DMA queue (wk first: K-projection
        # starts the kernel); the SP queue carries only input slabs so
        # the first K slab lands as early as possible.
        nc.scalar.dma_start(wk_sb[:], Wk)
        nc.scalar.dma_start(bk_sb[:], bk)
        nc.scalar.dma_start(on_sb[:], ones)
        nc.scalar.dma_start(wq_sb[:], Wq)
        nc.scalar.dma_start(wv_sb[:], Wv)
        nc.scalar.dma_start(wo_sb[:], WoT)
        nc.scalar.dma_start(bq_sb[:], bq)
        nc.scalar.dma_start(bv_sb[:], bv)

        # PE warm-up: dense back-to-back bf16 matmuls while the first DMAs
        # land, so the HAM un-throttles the PE clock before real work. The
        # warm-up exp also pre-loads the ACT exp table-set.
        with tc.tile_pool(name="wu", bufs=1) as wu_pool:
            wu = wu_pool.tile([128, 640], BF16)
            nc.gpsimd.memset(wu[:], 0.0)
            wux = wu_pool.tile([1, 32], F32, name="wux")
            nc.scalar.activation(wux[:], wu[0:1, 0:32], AF.Exp, scale=1.0)
            for i in range(28):
                wp = psF.tile([128, 512], F32, tag="psf", name="wup")
                nc.tensor.matmul(wp[:], wu[:, 0:128], wu[:, 128:640],
                                 start=True, stop=True)

        qt_sb = [persist.tile([128, S], BF16, tag=f"qt{j}", name=f"qt{j}")
                 for j in range(2)]
        kt_sb = [persist.tile([128, S], BF16, tag=f"kt{j}", name=f"kt{j}")
                 for j in range(2)]
        v_sb = persist.tile([128, NKT * 260], BF16, tag="v")

        # ---- projection emitters ----
        def emit_slab_dma(x_dram, c, pool=None, eng=None):
            slab = (pool or xt_pool).tile([128, T * 512], BF16, tag="slab")
            (eng or nc.sync).dma_start(
                slab[:], x_dram[:, c * T * 512 : (c + 1) * T * 512])
            return slab

        def emit_qk_steps(slab, w_sb, b_sb, dst, c, j):
            """One Q/K psum group as 3 filler steps (3+3+3 matmuls)."""
            ps = psF.tile([128, 512], F32, tag="psf", name="psqk")

            def step(t0, t1, ps=ps):
                for t in range(t0, t1):
                    nc.tensor.matmul(
                        ps[:],
                        w_sb[:, t * 256 + j * 128 : t * 256 + j * 128 + 128],
                        slab[:, t * 512 : (t + 1) * 512],
                        start=(t == 0), stop=False,
                    )
                if t1 == T:
                    nc.tensor.matmul(
                        ps[:], b_sb[:, j * 128 : (j + 1) * 128],
                        on_sb[:, 0:512], start=False, stop=True,
                    )
                    nc.vector.tensor_copy(
                        dst[:, c * 512 : (c + 1) * 512], ps[:]
                    )

            return [lambda a=a, b=b: step(a, b) for a, b in
                    [(0, 2), (2, 4), (4, 6), (6, T)]]

        def emit_v_steps(slab, c, s):
            """One V psum group split into 3 filler steps."""
            ps = psF.tile([128, 512], F32, tag="psf", name="psv")

            def step(t0, t1, ps=ps):
                for t in range(t0, t1):
                    nc.tensor.matmul(
                        ps[:, 0:260],
                        slab[:, t * 512 + s * 128 : t * 512 + s * 128 + 128],
                        wv_sb[:, t * 260 : (t + 1) * 260],
                        start=(t == 0), stop=False,
                    )
                if t1 == T:
                    nc.tensor.matmul(
                        ps[:, 0:260], on_sb[:, 0:128], bv_sb[:],
                        start=False, stop=True,
                    )
                    nc.vector.tensor_copy(
                        v_sb[:, (4 * c + s) * 260 : (4 * c + s + 1) * 260],
                        ps[:, 0:260],
                    )

            return [lambda a=a, b=b: step(a, b) for a, b in
                    [(0, 3), (3, 6), (6, T)]]

        # ---- preamble: K^T chunk 0 + Q^T chunk 0 head pair 0; the
        # rest (K^T 1-3, Q0 pair 1, all of V) is injected as fillers
        # into the first sweep, ordered against its consumption ----
        k0slab = emit_slab_dma(xkT, 0)
        k1slab = emit_slab_dma(xkT, 1)
        q0slab = emit_slab_dma(xqT, 0)
        for j in range(2):
            for f in emit_qk_steps(k0slab, wk_sb, bk_sb, kt_sb[j], 0, j):
                f()
        for f in emit_qk_steps(q0slab, wq_sb, bq_sb, qt_sb[0], 0, 0):
            f()

        # ---- fc_out group emitter (4 K=64 matmuls + drain + DMA) ----
        dma_q = [0]

        def emit_fc_group(qc, ot_tiles, ss, e2, o_sb, anchor=None,
                          act_drain=False):
            ps_f = psF.tile([128, 512], F32, tag="psf", name="psf")
            for h in range(4):
                mm = nc.tensor.matmul(
                    ps_f[:],
                    ot_tiles[h][:, ss * 128 : (ss + 1) * 128],
                    wo_sb[:, h * E + e2 * 512 : h * E + e2 * 512 + 512],
                    start=(h == 0), stop=(h == 3),
                )
                if h == 0 and anchor is not None:
                    tile.add_dep_helper(
                        mm.ins, anchor.ins,
                        reason="fc_out deferred behind later attention",
                    )
            nc.vector.tensor_copy(
                o_sb[:, e2 * 512 : (e2 + 1) * 512], ps_f[:]
            )
            if e2 == E2 - 1:
                r0 = qc * 512 + ss * 128
                nc.sync.dma_start(out[r0 : r0 + 128, :], o_sb[:])

        def emit_fc_half(qc, ot_tiles, ss, e2, o_sb, h0, anchor=None):
            """Half an fc_out group (2 heads). The h0==0 half can run
            during the final sweep (only the first head pair's ot is
            needed); the h0==2 half adds on top at the tail."""
            ps_f = psF.tile([128, 512], F32, tag="psf", name="psfh")
            for h in (h0, h0 + 1):
                mm = nc.tensor.matmul(
                    ps_f[:],
                    ot_tiles[h][:, ss * 128 : (ss + 1) * 128],
                    wo_sb[:, h * E + e2 * 512 : h * E + e2 * 512 + 512],
                    start=(h == h0), stop=(h == h0 + 1),
                )
                if h == h0 and anchor is not None:
                    tile.add_dep_helper(
                        mm.ins, anchor.ins,
                        reason="fc_out deferred behind later attention",
                    )
            osl = o_sb[:, e2 * 512 : (e2 + 1) * 512]
            if h0 == 0:
                nc.vector.tensor_copy(osl, ps_f[:])
            else:
                nc.vector.tensor_add(osl, osl, ps_f[:])
                if e2 == E2 - 1:
                    r0 = qc * 512 + ss * 128
                    # tail: both DMA queues in parallel, half rows each
                    nc.sync.dma_start(out[r0 : r0 + 64, :], o_sb[0:64, :])
                    nc.scalar.dma_start(out[r0 + 64 : r0 + 128, :],
                                        o_sb[64:128, :])

        # ---- phase B: attention sweeps, cross-sweep pipelined ----
        DEPTH = 5  # o-matmuls trail s/exp by this many k-tiles
        last_s_mm = [None]
        fillers = []      # pending filler closures (V/Q projections)
        fc_fillers = []   # pending fc groups (gated later: their ot
                          # inputs come from the previous sweep's
                          # normalize chain, ~5us of DVE/gpsimd latency)
        pending = []      # previous sweep's trailing o's + raw drains
        rest_queue = []   # deferred reciprocal/normalize chains

        def inject(n):
            k = min(n, len(fillers))
            for _ in range(k):
                fillers.pop(0)()
            return k

        def inject_fc(n):
            k = min(n, len(fc_fillers))
            for _ in range(k):
                fc_fillers.pop(0)()
            return k

        def keep_warm(n=2):
            """Dummy matmuls on resident tiles: deny the HAM an idle
            window when no real filler work is available this slot."""
            for _ in range(n):
                wp = psF.tile([128, 512], F32, tag="psf", name="warm")
                nc.tensor.matmul(wp[:], kt_sb[0][:, 0:128],
                                 qt_sb[0][:, 0:512], start=True, stop=True)

        # First-sweep filler inventory, ordered against consumption
        # (s needs K chunk c by slot 4c; o, DEPTH=8-delayed, needs V
        # chunk c by slot 4c+8; pair-1 work by the second sweep) and
        # against the slab rings' DMA readiness.
        k2slab = emit_slab_dma(xkT, 2)
        k3slab = emit_slab_dma(xkT, 3)
        # V slabs ride the ACT DMA queue (free after the weights) so
        # they land before the first sweep needs them
        vslabs = [emit_slab_dma(xvT, c, pool=xv_pool, eng=nc.scalar)
                  for c in range(C)]

        def v_units(c):
            us = []
            for sblk in range(4):
                us.extend(emit_v_steps(vslabs[c], c, sblk))
            return us

        fillers.extend(emit_qk_steps(k1slab, wk_sb, bk_sb, kt_sb[0], 1, 0))
        fillers.extend(emit_qk_steps(k2slab, wk_sb, bk_sb, kt_sb[0], 2, 0))
        fillers.extend(v_units(0))
        fillers.extend(emit_qk_steps(q0slab, wq_sb, bq_sb, qt_sb[1], 0, 1))
        fillers.extend(v_units(1))
        fillers.extend(emit_qk_steps(k1slab, wk_sb, bk_sb, kt_sb[1], 1, 1))
        fillers.extend(emit_qk_steps(k2slab, wk_sb, bk_sb, kt_sb[1], 2, 1))
        fillers.extend(v_units(2))
        fillers.extend(emit_qk_steps(k3slab, wk_sb, bk_sb, kt_sb[0], 3, 0))
        fillers.extend(emit_qk_steps(k3slab, wk_sb, bk_sb, kt_sb[1], 3, 1))
        fillers.extend(v_units(3))

        ot_store = {}
        sweeps = [(qc, j) for qc in range(QC) for j in range(2)]
        for w, (qc, j) in enumerate(sweeps):
            # filler inventory for this sweep
            if j == 0 and qc + 1 < QC:
                qslab = emit_slab_dma(xqT, qc + 1)
                for jj in range(2):
                    fillers.extend(
                        emit_qk_steps(qslab, wq_sb, bq_sb,
                                      qt_sb[jj], qc + 1, jj))
            if qc > 0:
                for ss in (0, 1) if j == 0 else (2, 3):
                    o_sb = os_pool.tile([128, E], F32, tag="osb",
                                        name="osb")
                    for e2 in range(E2):
                        fc_fillers.append(
                            lambda q=qc - 1, s=ss, e=e2, ob=o_sb:
                            emit_fc_group(q, ot_store[q], s, e, ob,
                                          anchor=last_s_mm[0]))
            if w == len(sweeps) - 1:
                # last chunk: the first head pair's fc halves can run
                # inside this sweep (their ot comes from this chunk's
                # j=0 normalize); the other halves add on at the tail
                tail_osb = []
                for ss in range(4):
                    o_sb = os_pool.tile([128, E], F32, tag="osb",
                                        name="osb")
                    tail_osb.append(o_sb)
                    for e2 in range(E2):
                        fc_fillers.append(
                            lambda s=ss, e=e2, ob=o_sb:
                            emit_fc_half(QC - 1, ot_store[QC - 1], s, e,
                                         ob, 0, anchor=last_s_mm[0]))

            po = [psB_o.tile([65, 512], F32, tag="po", name=f"po{e}")
                  for e in range(2)]
            pts = {}

            def emit_o(kt, po=po, pts=pts, j=j):
                for e in range(2):
                    h = 2 * j + e
                    nc.tensor.matmul(
                        po[e][:],
                        v_sb[:, kt * 260 + 65 * h : kt * 260 + 65 * h + 65],
                        pts[kt][:, e * 512 : (e + 1) * 512],
                        start=(kt == 0), stop=(kt == NKT - 1),
                    )
                del pts[kt]

            def make_normalize(po=po, qc=qc, j=j):
                """Split normalize: the raw drains pop right behind the
                trailing o's (freeing the po PSUM ring fast), while the
                slow reciprocal chain is deferred so it never convoys
                the DVE queue at a sweep boundary."""
                raws = []

                def norm_raw():
                    for e in range(2):
                        raw = raw_pool.tile([65, 512], F32, tag="raw",
                                            name="raw")
                        nc.vector.tensor_copy(raw[:], po[e][:])
                        raws.append(raw)

                def norm_rest():
                    ots = ot_store.setdefault(qc, {})
                    for e in range(2):
                        h = 2 * j + e
                        raw = raws[e]
                        rc = rc_pool.tile([1, 512], F32, tag="rc", name="rc")
                        with nc.allow_low_precision(reason="denom recip"):
                            nc.vector.reciprocal(rc[:], raw[64:65, :])
                        bc = bc_pool.tile([64, 512], F32, tag="bc", name="bc")
                        nc.gpsimd.partition_broadcast(bc[:], rc[:])
                        ot = ot_pool.tile([64, 512], F32R, tag="ot",
                                          name="ot")
                        nc.vector.tensor_mul(ot[:], raw[0:64, :], bc[:])
                        ots[h] = ot
                return norm_raw, norm_rest

            if w == 0:
                d = 8      # V chunks arrive as fillers; o trails more
            elif w == len(sweeps) - 1:
                d = 2      # shorten the final ACT-paced tail
            else:
                d = DEPTH
            for kt in range(NKT):
                ps_s = psB_s.tile([128, 1024], F32, tag="pss", name="pss")
                # the two heads of pair j land in disjoint PE row
                # groups (base partitions 0 / 64) and disjoint PSUM
                # banks, so these run concurrently in the array
                for e in range(2):
                    last_s_mm[0] = nc.tensor.matmul(
                        ps_s[:, e * 512 : (e + 1) * 512],
                        kt_sb[j][64 * e : 64 * e + 64,
                                 kt * 128 : (kt + 1) * 128],
                        qt_sb[j][64 * e : 64 * e + 64,
                                 qc * 512 : (qc + 1) * 512],
                        start=True, stop=True,
                    )
                pt = pt_pool.tile([128, 1024], BF16, tag="pt", name="pt")
                nc.scalar.activation(pt[:], ps_s[:], AF.Exp, scale=scale)
                pts[kt] = pt
                if kt < d:
                    for _ in range(2 if pending else 0):
                        if pending:
                            pending.pop(0)()
                else:
                    while pending:   # raw drains must precede o(0)'s WAR
                        pending.pop(0)()
                    emit_o(kt - d)
                if kt == 6 and rest_queue:
                    rest_queue.pop(0)()
                inject(5 if w == 0 and kt < 12 else (4 if w == 0 else 2))
                if j == 0 or w == len(sweeps) - 1:
                    if kt >= 10:
                        inject_fc(2)
                elif kt >= 2:
                    inject_fc(1)

            pending = [lambda kt=kt, f=emit_o: f(kt)
                       for kt in range(NKT - d, NKT)]
            if w < len(sweeps) - 1:
                nr, nrest = make_normalize()
                pending.append(nr)
                rest_queue.append(nrest)
            else:
                last_norm = (po, qc, j)

        # tail: flush trailing work, then the last pair's normalize in
        # 128-column blocks interleaved with its fc halves, so the fc
        # matmuls start ~1.5us after the last exp instead of waiting
        # for two full-width 3.3us reciprocals
        while pending:
            pending.pop(0)()
        while rest_queue:
            rest_queue.pop(0)()
        while fillers:
            inject(len(fillers))
        while fc_fillers:
            inject_fc(len(fc_fillers))
        po_l, qc_l, j_l = last_norm
        ots_l = ot_store.setdefault(qc_l, {})
        raws_l = []
        for e in range(2):
            raw = raw_pool.tile([65, 512], F32, tag="raw", name="rawt")
            nc.vector.tensor_copy(raw[:], po_l[e][:])
            raws_l.append(raw)
            ots_l[2 * j_l + e] = ot_pool.tile([64, 512], F32R, tag="ot",
                                              name="ott")
        for ss in range(4):
            cs = slice(ss * 128, ss * 128 + 128)
            for e in range(2):
                rcb = rc_pool.tile([1, 128], F32, tag="rcb", name="rcb")
                with nc.allow_low_precision(reason="denom recip"):
                    nc.vector.reciprocal(rcb[:], raws_l[e][64:65, cs])
                bcb = bc_pool.tile([64, 128], F32, tag="bcb", name="bcb")
                nc.gpsimd.partition_broadcast(bcb[:], rcb[:])
                nc.vector.tensor_mul(ots_l[2 * j_l + e][:, cs],
                                     raws_l[e][0:64, cs], bcb[:])
            for e2 in range(E2):
                emit_fc_half(QC - 1, ots_l, ss, e2, tail_osb[ss], 2)

    nc.compile()
    return nc

_NC_CACHE = [None]


def _get_nc():
    if _NC_CACHE[0] is None:
        _NC_CACHE[0] = build_nc(S=S, E=E)
    return _NC_CACHE[0]


def _pack_w_ktile(W):
    """[E, F] -> [128, (E//128)*F] with k-tile-major packing: column
    block t holds W[t*128:(t+1)*128, :]."""
    E_, F_ = W.shape
    T_ = E_ // 128
    return np.ascontiguousarray(
        W.reshape(T_, 128, F_).transpose(1, 0, 2).reshape(128, T_ * F_))


def _pack_x(xT, S=2048, E=1024):
    """[E, S] f32 -> [128, C*T*512] chunk-major so each 512-seq slab is
    one contiguous [128, T*512] DMA."""
    T_ = E // 128
    C_ = S // 512
    return np.ascontiguousarray(
        xT.reshape(T_, 128, C_, 512).transpose(1, 2, 0, 3).reshape(
            128, C_ * T_ * 512))


def make_in_maps(query, key, value, Wq, bq, Wk, bk, Wv, bv, Wo, bo):
    """Shard the full inputs into the 8 per-core input dicts."""
    bf = mybir.dt.np(BF16)
    f32 = np.float32
    query = np.asarray(query, f32)
    key = np.asarray(key, f32)
    value = np.asarray(value, f32)
    Wq, bq = np.asarray(Wq, f32), np.asarray(bq, f32)
    Wk, bk = np.asarray(Wk, f32), np.asarray(bk, f32)
    Wv, bv = np.asarray(Wv, f32), np.asarray(bv, f32)
    Wo, bo = np.asarray(Wo, f32), np.asarray(bo, f32)

    xT = {}
    for b in range(B):
        xT[b] = (
            _pack_x(query[b].T).astype(bf),
            _pack_x(key[b].T).astype(bf),
            _pack_x(value[b].T).astype(bf),
        )

    ones = np.ones((1, 512), bf)
    in_maps = []
    for c in range(N_CORES):
        b, g = c // 4, c % 4
        fs = slice(FL * g, FL * g + FL)
        # projection weights: reference computes x @ W.T, so the device
        # weight matrix is W.T's column slice = W[fs, :].T  [E, FL]
        wq_c = np.ascontiguousarray(Wq[fs, :].T)
        wk_c = np.ascontiguousarray(Wk[fs, :].T)
        wv_c = np.ascontiguousarray(Wv[fs, :].T)
        # V with interleaved ones-columns (via the bias row)
        wv_pack = np.zeros((E, HL * 65), f32)
        bv_pack = np.zeros((1, HL * 65), f32)
        bq_c = bq[fs][None, :]
        bk_c = bk[fs][None, :]
        for h in range(HL):
            wv_pack[:, 65 * h : 65 * h + 64] = wv_c[:, 64 * h : 64 * h + 64]
            bv_pack[0, 65 * h : 65 * h + 64] = bv[fs][64 * h : 64 * h + 64]
            bv_pack[0, 65 * h + 64] = 1.0
        # fc_out rows for the local features (bias bo is added on the
        # host during the cross-core reduction)
        wot = np.zeros((64, HL * E), f32)
        for h in range(HL):
            wot[:, E * h : E * h + E] = Wo[:, FL * g + 64 * h : FL * g + 64 * h + 64].T
        in_maps.append({
            "xqT": xT[b][0], "xkT": xT[b][1], "xvT": xT[b][2],
            "Wq": _pack_w_ktile(wq_c).astype(bf),
            "Wk": _pack_w_ktile(wk_c).astype(bf),
            "Wv": _pack_w_ktile(wv_pack).astype(bf),
            "bq": bq_c.astype(bf), "bk": bk_c.astype(bf),
            "bv": bv_pack.astype(bf),
            "WoT": wot,
            "ones": ones,
        })
    return in_maps


def assemble_output(results, bo):
    """Sum the row-parallel partial fc_out results per batch, add bo."""
    out = np.empty((B, S, E), np.float32)
    for b in range(B):
        acc = results[4 * b]["out"].astype(np.float32).copy()
        for g in range(1, 4):
            acc += results[4 * b + g]["out"]
        out[b] = acc + bo[None, :]
    return out


def kernel(query, key, value, Wq, bq, Wk, bk, Wv, bv, Wo, bo, **run_kwargs):
    nc = _get_nc()
    in_maps = make_in_maps(query, key, value, Wq, bq, Wk, bk, Wv, bv, Wo, bo)
    res = run_bass_kernel_spmd(nc, in_maps, core_ids=list(range(N_CORES)),
                               **run_kwargs)
    out = assemble_output(res.results, np.asarray(bo, np.float32))
    kernel.last_result = res
    return out
